# revision 10
# baseline (speedup 1.0000x reference)
"""Trainium2 Bass kernel for the dual-branch CustomLSTMCell.

Math (reference):
    hx_l = [h_light | y]  [B, H+I]     hx_t = [h_temp | y]
    z_br = hx_br @ W_br.T + b_br       (W_br = vstack(w_f,w_i,w_c,w_o) [4H, H+I])
    f,i,ch,o = sigmoid/sigmoid/tanh/sigmoid splits of z_br
    c_new = (f1 + f2) * c_light + i1*ch1 + i2*ch2      (c_temp is unused)
    h_new = (o1 + o2) * tanh(c_new)

Strategy: pure data-parallel over 8 NeuronCores — batch 4096 -> 8 x 512,
weights replicated. Per core we compute z.T tiles: psum[zcol 128, batch 512]
= Wtile[K=128, M=128].T @ hxT[K=128, N=512], accumulated over K=1536 (12
k-tiles), in fp16 (1 PE cycle/row like bf16 — fp32 would be 4x slower — but
with 8x finer mantissa). Gate bias + sigmoid/tanh run on the Scalar engine
straight out of PSUM (bias is per-partition in this transposed layout), the
LSTM cell elementwise runs on the Vector engine in fp32, results DMA out
transposed, and the host un-transposes. All transposes/casts happen host-side
so every device DMA is a contiguous 2D partition-major stream.

Schedule notes: weight DMAs are issued with a software prefetch distance of
PREFETCH tiles so the first matmul's operands land within ~2us of kernel
start; the per-branch gate order is (i, c, f, o) so the output gate of the
temp branch - the only input of the final h_new chain - finishes last and
the tail after the final matmul is short.

The DMA rings are specialized so the weight stream is never queued behind
bulk activation loads (the profile showed the PE starving at ~12us and the
HAM clock-gate re-throttling as a result): sync carries only weight tiles
(plus wt0 on scalar), gpsimd carries bias + a_l + a_t + c_light + most
outputs. The last gate's output chain is split into two batch
halves so the activation/mul/add/DMA tail pipelines against the final
matmul block.
"""

import os
import sys

for _p in ("/opt/trn_rl_repo",):
    if os.path.isdir(_p) and _p not in sys.path:
        sys.path.append(_p)

import numpy as np

import concourse.bass as bass
import concourse.mybir as mybir
import concourse.tile as tile
from concourse import bacc
from concourse.bass_utils import run_bass_kernel_spmd

B, I, H = 4096, 512, 1024
N_CORES = 8
BS = B // N_CORES          # 512 batch rows per core
K = H + I                  # 1536 contraction
KT = K // 128              # 12 k-tiles
RT = H // 128              # 8 zcol (hidden) tiles per gate
N_W = RT * 2 * 4           # 64 weight tiles: (r, branch, gate)
GATE_ORDER = (1, 2, 0, 3)  # i, c, f, o
PREFETCH = 8               # weight tiles in flight ahead of use

_F32 = mybir.dt.float32
_F16 = mybir.dt.float16
AF = mybir.ActivationFunctionType
F16 = np.float16


def _build_nc():
    nc = bacc.Bacc("TRN2", target_bir_lowering=False, debug=False,
                   enable_asserts=False)

    wp = nc.dram_tensor("wp", [N_W, 128, KT * 128], _F16, kind="ExternalInput")
    a_l = nc.dram_tensor("a_l", [128, KT * BS], _F16, kind="ExternalInput")
    a_t = nc.dram_tensor("a_t", [128, KT * BS], _F16, kind="ExternalInput")
    bp = nc.dram_tensor("bp", [128, N_W], _F32, kind="ExternalInput")
    ct = nc.dram_tensor("ct", [RT, 128, BS], _F32, kind="ExternalInput")
    h_out = nc.dram_tensor("h_out", [RT, 128, BS], _F32, kind="ExternalOutput")
    c_out = nc.dram_tensor("c_out", [RT, 128, BS], _F32, kind="ExternalOutput")

    with tile.TileContext(nc) as tc:
        with (
            tc.tile_pool(name="const", bufs=1) as cpool,
            tc.tile_pool(name="w", bufs=PREFETCH + 4) as wpool,
            tc.tile_pool(name="gates", bufs=16) as gpool,
            tc.tile_pool(name="cin", bufs=2) as cin_pool,
            tc.tile_pool(name="ew", bufs=4) as epool,
            tc.tile_pool(name="out", bufs=4) as opool,
            tc.tile_pool(name="psum", bufs=8, space="PSUM") as pspool,
        ):
            wt_tiles = {}

            def issue_wt(seq, eng=None):
                t = wpool.tile([128, KT * 128], _F16, tag="w")
                (eng or nc.sync).dma_start(out=t[:], in_=wp[seq])
                wt_tiles[seq] = t

            # PE pre-warm: dummy matmuls on a zeroed tile start the HAM
            # clock-gate busy window while the first operands are in flight.
            # Just enough to cover the first weight tile's DMA (~1.3us) —
            # beyond that, cold *real* matmuls warm the window doing useful
            # work at half rate instead of none.
            warm = cpool.tile([128, BS], _F16, tag="warm")
            nc.gpsimd.memset(warm[:], 0.0)
            wpsum = pspool.tile([128, BS], _F32, tag="pt")
            for _ in range(3):
                nc.tensor.matmul(wpsum[:], warm[:, 0:128], warm[:],
                                 start=True, stop=True)

            # startup: the weight stream owns the sync ring exclusively;
            # activations/bias/c_light ride the other engines' rings so the
            # PE's weight feed is never queued behind bulk loads.
            issue_wt(0, nc.scalar)
            a_sb = []
            for name, src in (("al", a_l), ("at", a_t)):
                t = cpool.tile([128, KT * BS], _F16, tag=name)
                a_sb.append(t)
            bias_sb = cpool.tile([128, N_W], _F32, tag="bias")
            nc.gpsimd.dma_start(out=bias_sb[:], in_=bp[:])
            for k in range(KT):
                nc.gpsimd.dma_start(out=a_sb[0][:, bass.ts(k, BS)],
                                    in_=a_l[:, bass.ts(k, BS)])
            for k in range(1, PREFETCH):
                issue_wt(k)
            for k in range(KT):
                nc.gpsimd.dma_start(out=a_sb[1][:, bass.ts(k, BS)],
                                    in_=a_t[:, bass.ts(k, BS)])

            seq = 0  # sequential weight-tile index (matches host pack order)
            for r in range(RT):
                last_r = r == RT - 1
                ct_t = cin_pool.tile([128, BS], _F32, tag="ct")
                nc.gpsimd.dma_start(out=ct_t[:], in_=ct[r])

                gates = {}
                for br in range(2):
                    for g in GATE_ORDER:
                        if seq + PREFETCH < N_W:
                            issue_wt(seq + PREFETCH)
                        idx = (r * 2 + br) * 4 + g
                        wt = wt_tiles.pop(seq)
                        last_gate = last_r and br == 1 and g == 3
                        if last_gate:
                            # Split the final gate's matmuls into batch
                            # halves so the sigmoid/mul/add/DMA tail of the
                            # first half overlaps the second half's matmuls.
                            pts = [pspool.tile([128, BS // 2], _F32, tag="pt",
                                               name=f"pt_half{h}")
                                   for h in range(2)]
                            for h in range(2):
                                for k in range(KT):
                                    nc.tensor.matmul(
                                        pts[h][:],
                                        wt[:, bass.ts(k, 128)],
                                        a_sb[br][:, k * BS + h * (BS // 2):
                                                  k * BS + (h + 1) * (BS // 2)],
                                        start=(k == 0),
                                        stop=(k == KT - 1),
                                    )
                            gt = gpool.tile([128, BS], _F32, tag="gate")
                            for h in range(2):
                                nc.scalar.activation(
                                    gt[:, bass.ts(h, BS // 2)], pts[h][:],
                                    AF.Sigmoid,
                                    bias=bias_sb[:, idx:idx + 1], scale=1.0)
                        else:
                            pt = pspool.tile([128, BS], _F32, tag="pt")
                            for k in range(KT):
                                nc.tensor.matmul(
                                    pt[:],
                                    wt[:, bass.ts(k, 128)],
                                    a_sb[br][:, bass.ts(k, BS)],
                                    start=(k == 0),
                                    stop=(k == KT - 1),
                                )
                            gt = gpool.tile([128, BS], _F32, tag="gate")
                            func = AF.Tanh if g == 2 else AF.Sigmoid
                            nc.scalar.activation(gt[:], pt[:], func,
                                                 bias=bias_sb[:, idx:idx + 1],
                                                 scale=1.0)
                        gates[(br, g)] = gt
                        seq += 1

                f1, i1, ch1, o1 = (gates[(0, g)] for g in range(4))
                f2, i2, ch2, o2 = (gates[(1, g)] for g in range(4))

                t_a = epool.tile([128, BS], _F32, tag="ta")
                t_b = epool.tile([128, BS], _F32, tag="tb")
                t_c = epool.tile([128, BS], _F32, tag="tc")
                c_new = opool.tile([128, BS], _F32, tag="cn")
                nc.vector.tensor_mul(t_b[:], i1[:], ch1[:])
                nc.vector.tensor_mul(t_c[:], i2[:], ch2[:])
                nc.vector.tensor_add(t_b[:], t_b[:], t_c[:])
                nc.vector.tensor_add(t_a[:], f1[:], f2[:])        # f1+f2
                nc.vector.tensor_mul(t_a[:], t_a[:], ct_t[:])     # *c_light
                nc.vector.tensor_add(c_new[:], t_a[:], t_b[:])
                nc.gpsimd.dma_start(out=c_out[r], in_=c_new[:])

                th = epool.tile([128, BS], _F32, tag="th")
                nc.scalar.activation(th[:], c_new[:], AF.Tanh)
                h_new = opool.tile([128, BS], _F32, tag="hn")
                if last_r:
                    # o1*th runs during o2's matmuls; per-half o2*th + add
                    # + DMA pipeline against the second half's matmuls.
                    nc.vector.tensor_mul(t_b[:], o1[:], th[:])
                    for h in range(2):
                        sl = bass.ts(h, BS // 2)
                        nc.vector.tensor_mul(t_c[:, sl], o2[:, sl], th[:, sl])
                        nc.vector.tensor_add(h_new[:, sl], t_b[:, sl],
                                             t_c[:, sl])
                        eng = nc.sync if h == 0 else nc.gpsimd
                        eng.dma_start(
                            out=h_out[r, :, h * (BS // 2):(h + 1) * (BS // 2)],
                            in_=h_new[:, sl])
                else:
                    nc.vector.tensor_add(t_a[:], o1[:], o2[:])    # o1+o2
                    nc.vector.tensor_mul(h_new[:], t_a[:], th[:])
                    nc.gpsimd.dma_start(out=h_out[r], in_=h_new[:])

    nc.compile()
    return nc


_NC_CACHE = None


def _get_nc():
    global _NC_CACHE
    if _NC_CACHE is None:
        _NC_CACHE = _build_nc()
    return _NC_CACHE


def _pack_weights(inputs):
    """-> wp [N_W, 128, KT*128] f16, bp [128, N_W] f32 (shared by all cores).

    Weight-tile seq order must match the device loop: (r, br, g in GATE_ORDER).
    """
    wps, bps = [], []
    for suffix in ("_light", "_light_temp"):
        Wc = np.concatenate([inputs["w_f" + suffix], inputs["w_i" + suffix],
                             inputs["w_c" + suffix], inputs["w_o" + suffix]],
                            axis=0)                       # [4H, K]
        bc = np.concatenate([inputs["b_f" + suffix], inputs["b_i" + suffix],
                             inputs["b_c" + suffix], inputs["b_o" + suffix]])
        # tile (r, g): sbuf[kk, k*128 + m] = lhsT_k[kk, m]
        #            = Wc[g*1024 + r*128 + m, k*128 + kk]
        Wt = Wc.reshape(4, RT, 128, KT, 128)              # [g, r, m, k, kk]
        Wt = Wt.transpose(1, 0, 4, 3, 2)                  # [r, g, kk, k, m]
        wps.append(np.ascontiguousarray(Wt).astype(F16))
        bps.append(bc.reshape(4, RT, 128).transpose(2, 1, 0))  # [p, r, g]
    wp = np.stack(wps, axis=1)                            # [r, br, g, kk, k, m]
    wp = wp[:, :, GATE_ORDER]                             # device consumption order
    wp = np.ascontiguousarray(wp).reshape(N_W, 128, KT * 128)
    bp = np.stack(bps, axis=2)                            # [p, r, br, g]
    bp = np.ascontiguousarray(bp).reshape(128, N_W).astype(np.float32)
    return wp, bp


def _pack_core_inputs(inputs, wp, bp, core):
    sl = slice(core * BS, (core + 1) * BS)
    y = inputs["y"][sl]
    out = {"wp": wp, "bp": bp}
    for name, h in (("a_l", inputs["h_light"][sl]), ("a_t", inputs["h_temp"][sl])):
        hx = np.concatenate([h, y], axis=1).astype(F16)   # [BS, K]
        # sbuf[p, k*BS + j] = hx[j, k*128 + p]
        a2 = hx.reshape(BS, KT, 128).transpose(2, 1, 0)
        out[name] = np.ascontiguousarray(a2).reshape(128, KT * BS)
    cl = np.ascontiguousarray(inputs["c_light"][sl].astype(np.float32).T)
    out["ct"] = cl.reshape(RT, 128, BS)
    return out


def make_in_maps(**inputs):
    wp, bp = _pack_weights(inputs)
    return [_pack_core_inputs(inputs, wp, bp, c) for c in range(N_CORES)]


def unpack_results(results):
    h_parts, c_parts = [], []
    for res in results:
        h_parts.append(res["h_out"].reshape(H, BS).T)
        c_parts.append(res["c_out"].reshape(H, BS).T)
    h_new = np.ascontiguousarray(np.concatenate(h_parts, axis=0), dtype=np.float32)
    c_new = np.ascontiguousarray(np.concatenate(c_parts, axis=0), dtype=np.float32)
    return h_new, c_new


def kernel(**inputs):
    inputs = {k: np.asarray(v) for k, v in inputs.items()}
    nc = _get_nc()
    in_maps = make_in_maps(**inputs)
    res = run_bass_kernel_spmd(nc, in_maps, list(range(N_CORES)))
    return unpack_results(res.results)



# revision 15
# speedup vs baseline: 1.0423x; 1.0423x over previous
"""Trainium2 Bass kernel for the dual-branch CustomLSTMCell.

Math (reference):
    hx_l = [h_light | y]  [B, H+I]     hx_t = [h_temp | y]
    z_br = hx_br @ W_br.T + b_br       (W_br = vstack(w_f,w_i,w_c,w_o) [4H, H+I])
    f,i,ch,o = sigmoid/sigmoid/tanh/sigmoid splits of z_br
    c_new = (f1 + f2) * c_light + i1*ch1 + i2*ch2      (c_temp is unused)
    h_new = (o1 + o2) * tanh(c_new)

Strategy: pure data-parallel over 8 NeuronCores — batch 4096 -> 8 x 512,
weights replicated. Per core we compute z.T tiles: psum[zcol 128, batch 512]
= Wtile[K=128, M=128].T @ hxT[K=128, N=512], accumulated over K=1536 (12
k-tiles), in fp16 (1 PE cycle/row like bf16 — fp32 would be 4x slower — but
with 8x finer mantissa). Gate bias + sigmoid/tanh run on the Scalar engine
straight out of PSUM (bias is per-partition in this transposed layout), the
LSTM cell elementwise runs on the Vector engine in fp32, results DMA out
transposed, and the host un-transposes. All transposes/casts happen host-side
so every device DMA is a contiguous 2D partition-major stream.

Schedule notes: weight DMAs are issued with a software prefetch distance of
PREFETCH tiles so the first matmul's operands land within ~2us of kernel
start; the per-branch gate order is (i, c, f, o) so the output gate of the
temp branch - the only input of the final h_new chain - finishes last and
the tail after the final matmul is short.

DMA schedule: the early weight tiles are interleaved INTO the activation
stream on the sync ring (one weight tile every two activation k-tiles) so
the PE is never starved of weights while the 3.1MB of activations loads —
the unmodified baseline profile showed exactly that starvation at ~12us,
with the HAM clock-gate re-throttling as a result. bias/c_light ride the
scalar ring (descriptor pushes before the first activations). Outputs stay
on the hardware DGE rings (sync/scalar) — gpsimd's software DGE measured
~5x slower and stretched the tail. The last gate's output chain is split
into two batch halves so the sigmoid/mul/add/DMA tail pipelines against
the final matmul block.
"""

import os
import sys

for _p in ("/opt/trn_rl_repo",):
    if os.path.isdir(_p) and _p not in sys.path:
        sys.path.append(_p)

import numpy as np

import concourse.bass as bass
import concourse.mybir as mybir
import concourse.tile as tile
from concourse import bacc
from concourse.bass_utils import run_bass_kernel_spmd

B, I, H = 4096, 512, 1024
N_CORES = 8
BS = B // N_CORES          # 512 batch rows per core
K = H + I                  # 1536 contraction
KT = K // 128              # 12 k-tiles
RT = H // 128              # 8 zcol (hidden) tiles per gate
N_W = RT * 2 * 4           # 64 weight tiles: (r, branch, gate)
GATE_ORDER = (1, 2, 0, 3)  # i, c, f, o
PREFETCH = 8               # weight tiles in flight ahead of use

_F32 = mybir.dt.float32
_F16 = mybir.dt.float16
AF = mybir.ActivationFunctionType
F16 = np.float16


def _build_nc():
    nc = bacc.Bacc("TRN2", target_bir_lowering=False, debug=False,
                   enable_asserts=False)

    wp = nc.dram_tensor("wp", [N_W, 128, KT * 128], _F16, kind="ExternalInput")
    a_l = nc.dram_tensor("a_l", [128, KT * BS], _F16, kind="ExternalInput")
    a_t = nc.dram_tensor("a_t", [128, KT * BS], _F16, kind="ExternalInput")
    bp = nc.dram_tensor("bp", [128, N_W], _F32, kind="ExternalInput")
    ct = nc.dram_tensor("ct", [RT, 128, BS], _F32, kind="ExternalInput")
    h_out = nc.dram_tensor("h_out", [RT, 128, BS], _F32, kind="ExternalOutput")
    c_out = nc.dram_tensor("c_out", [RT, 128, BS], _F32, kind="ExternalOutput")

    with tile.TileContext(nc) as tc:
        with (
            tc.tile_pool(name="const", bufs=1) as cpool,
            tc.tile_pool(name="w", bufs=PREFETCH + 4) as wpool,
            tc.tile_pool(name="gates", bufs=16) as gpool,
            tc.tile_pool(name="cin", bufs=2) as cin_pool,
            tc.tile_pool(name="ew", bufs=4) as epool,
            tc.tile_pool(name="out", bufs=4) as opool,
            tc.tile_pool(name="psum", bufs=8, space="PSUM") as pspool,
        ):
            wt_tiles = {}

            def issue_wt(seq, eng=None):
                t = wpool.tile([128, KT * 128], _F16, tag="w")
                (eng or nc.sync).dma_start(out=t[:], in_=wp[seq])
                wt_tiles[seq] = t

            # PE pre-warm: dummy matmuls on a zeroed tile start the HAM
            # clock-gate busy window while the first operands are in flight.
            warm = cpool.tile([128, BS], _F16, tag="warm")
            nc.gpsimd.memset(warm[:], 0.0)
            wpsum = pspool.tile([128, BS], _F32, tag="pt")
            for _ in range(8):
                nc.tensor.matmul(wpsum[:], warm[:, 0:128], warm[:],
                                 start=True, stop=True)

            # startup: wt0 moves alone on the scalar ring; the sync ring
            # interleaves one weight tile per two activation k-tiles so
            # neither stream starves the PE; a_l's k0 tile rides gpsimd so
            # the very first matmul's rhs doesn't queue at all.
            issue_wt(0, nc.scalar)
            a_sb = []
            for name, src in (("al", a_l), ("at", a_t)):
                t = cpool.tile([128, KT * BS], _F16, tag=name)
                a_sb.append(t)
            bias_sb = cpool.tile([128, N_W], _F32, tag="bias")
            nc.scalar.dma_start(out=bias_sb[:], in_=bp[:])
            nc.gpsimd.dma_start(out=a_sb[0][:, bass.ts(0, BS)],
                                in_=a_l[:, bass.ts(0, BS)])
            nwt = 1

            def startup_wt():
                nonlocal nwt
                if nwt < PREFETCH:
                    issue_wt(nwt)
                    nwt += 1

            startup_wt()
            for k in range(1, KT):
                nc.sync.dma_start(out=a_sb[0][:, bass.ts(k, BS)],
                                  in_=a_l[:, bass.ts(k, BS)])
                if k % 2 == 0:
                    startup_wt()
            for k in range(KT):
                nc.sync.dma_start(out=a_sb[1][:, bass.ts(k, BS)],
                                  in_=a_t[:, bass.ts(k, BS)])
                if k % 2 == 1:
                    startup_wt()
            while nwt < PREFETCH:
                startup_wt()

            seq = 0  # sequential weight-tile index (matches host pack order)
            for r in range(RT):
                last_r = r == RT - 1
                ct_t = cin_pool.tile([128, BS], _F32, tag="ct")
                nc.scalar.dma_start(out=ct_t[:], in_=ct[r])

                gates = {}
                for br in range(2):
                    for g in GATE_ORDER:
                        if seq + PREFETCH < N_W:
                            issue_wt(seq + PREFETCH)
                        idx = (r * 2 + br) * 4 + g
                        wt = wt_tiles.pop(seq)
                        last_gate = last_r and br == 1 and g == 3
                        if last_gate:
                            # Split the final gate's matmuls into batch
                            # halves so the sigmoid/mul/add/DMA tail of the
                            # first half overlaps the second half's matmuls.
                            pts = [pspool.tile([128, BS // 2], _F32, tag="pt",
                                               name=f"pt_half{h}")
                                   for h in range(2)]
                            for h in range(2):
                                for k in range(KT):
                                    nc.tensor.matmul(
                                        pts[h][:],
                                        wt[:, bass.ts(k, 128)],
                                        a_sb[br][:, k * BS + h * (BS // 2):
                                                  k * BS + (h + 1) * (BS // 2)],
                                        start=(k == 0),
                                        stop=(k == KT - 1),
                                    )
                            gt = gpool.tile([128, BS], _F32, tag="gate")
                            for h in range(2):
                                nc.scalar.activation(
                                    gt[:, bass.ts(h, BS // 2)], pts[h][:],
                                    AF.Sigmoid,
                                    bias=bias_sb[:, idx:idx + 1], scale=1.0)
                        else:
                            pt = pspool.tile([128, BS], _F32, tag="pt")
                            for k in range(KT):
                                nc.tensor.matmul(
                                    pt[:],
                                    wt[:, bass.ts(k, 128)],
                                    a_sb[br][:, bass.ts(k, BS)],
                                    start=(k == 0),
                                    stop=(k == KT - 1),
                                )
                            gt = gpool.tile([128, BS], _F32, tag="gate")
                            func = AF.Tanh if g == 2 else AF.Sigmoid
                            nc.scalar.activation(gt[:], pt[:], func,
                                                 bias=bias_sb[:, idx:idx + 1],
                                                 scale=1.0)
                        gates[(br, g)] = gt
                        seq += 1

                f1, i1, ch1, o1 = (gates[(0, g)] for g in range(4))
                f2, i2, ch2, o2 = (gates[(1, g)] for g in range(4))

                t_a = epool.tile([128, BS], _F32, tag="ta")
                t_b = epool.tile([128, BS], _F32, tag="tb")
                t_c = epool.tile([128, BS], _F32, tag="tc")
                c_new = opool.tile([128, BS], _F32, tag="cn")
                nc.vector.tensor_mul(t_b[:], i1[:], ch1[:])
                nc.vector.tensor_mul(t_c[:], i2[:], ch2[:])
                nc.vector.tensor_add(t_b[:], t_b[:], t_c[:])
                nc.vector.tensor_add(t_a[:], f1[:], f2[:])        # f1+f2
                nc.vector.tensor_mul(t_a[:], t_a[:], ct_t[:])     # *c_light
                nc.vector.tensor_add(c_new[:], t_a[:], t_b[:])
                nc.scalar.dma_start(out=c_out[r], in_=c_new[:])

                th = epool.tile([128, BS], _F32, tag="th")
                nc.scalar.activation(th[:], c_new[:], AF.Tanh)
                h_new = opool.tile([128, BS], _F32, tag="hn")
                if last_r:
                    # o1*th runs during o2's matmuls; per-half o2*th + add
                    # + DMA pipeline against the second half's matmuls.
                    nc.vector.tensor_mul(t_b[:], o1[:], th[:])
                    for h in range(2):
                        sl = bass.ts(h, BS // 2)
                        nc.vector.tensor_mul(t_c[:, sl], o2[:, sl], th[:, sl])
                        nc.vector.tensor_add(h_new[:, sl], t_b[:, sl],
                                             t_c[:, sl])
                        eng = nc.sync if h == 0 else nc.scalar
                        eng.dma_start(
                            out=h_out[r, :, h * (BS // 2):(h + 1) * (BS // 2)],
                            in_=h_new[:, sl])
                else:
                    nc.vector.tensor_add(t_a[:], o1[:], o2[:])    # o1+o2
                    nc.vector.tensor_mul(h_new[:], t_a[:], th[:])
                    nc.sync.dma_start(out=h_out[r], in_=h_new[:])

    nc.compile()
    return nc


_NC_CACHE = None


def _get_nc():
    global _NC_CACHE
    if _NC_CACHE is None:
        _NC_CACHE = _build_nc()
    return _NC_CACHE


def _pack_weights(inputs):
    """-> wp [N_W, 128, KT*128] f16, bp [128, N_W] f32 (shared by all cores).

    Weight-tile seq order must match the device loop: (r, br, g in GATE_ORDER).
    """
    wps, bps = [], []
    for suffix in ("_light", "_light_temp"):
        Wc = np.concatenate([inputs["w_f" + suffix], inputs["w_i" + suffix],
                             inputs["w_c" + suffix], inputs["w_o" + suffix]],
                            axis=0)                       # [4H, K]
        bc = np.concatenate([inputs["b_f" + suffix], inputs["b_i" + suffix],
                             inputs["b_c" + suffix], inputs["b_o" + suffix]])
        # tile (r, g): sbuf[kk, k*128 + m] = lhsT_k[kk, m]
        #            = Wc[g*1024 + r*128 + m, k*128 + kk]
        Wt = Wc.reshape(4, RT, 128, KT, 128)              # [g, r, m, k, kk]
        Wt = Wt.transpose(1, 0, 4, 3, 2)                  # [r, g, kk, k, m]
        wps.append(np.ascontiguousarray(Wt).astype(F16))
        bps.append(bc.reshape(4, RT, 128).transpose(2, 1, 0))  # [p, r, g]
    wp = np.stack(wps, axis=1)                            # [r, br, g, kk, k, m]
    wp = wp[:, :, GATE_ORDER]                             # device consumption order
    wp = np.ascontiguousarray(wp).reshape(N_W, 128, KT * 128)
    bp = np.stack(bps, axis=2)                            # [p, r, br, g]
    bp = np.ascontiguousarray(bp).reshape(128, N_W).astype(np.float32)
    return wp, bp


def _pack_core_inputs(inputs, wp, bp, core):
    sl = slice(core * BS, (core + 1) * BS)
    y = inputs["y"][sl]
    out = {"wp": wp, "bp": bp}
    for name, h in (("a_l", inputs["h_light"][sl]), ("a_t", inputs["h_temp"][sl])):
        hx = np.concatenate([h, y], axis=1).astype(F16)   # [BS, K]
        # sbuf[p, k*BS + j] = hx[j, k*128 + p]
        a2 = hx.reshape(BS, KT, 128).transpose(2, 1, 0)
        out[name] = np.ascontiguousarray(a2).reshape(128, KT * BS)
    cl = np.ascontiguousarray(inputs["c_light"][sl].astype(np.float32).T)
    out["ct"] = cl.reshape(RT, 128, BS)
    return out


def make_in_maps(**inputs):
    wp, bp = _pack_weights(inputs)
    return [_pack_core_inputs(inputs, wp, bp, c) for c in range(N_CORES)]


def unpack_results(results):
    h_parts, c_parts = [], []
    for res in results:
        h_parts.append(res["h_out"].reshape(H, BS).T)
        c_parts.append(res["c_out"].reshape(H, BS).T)
    h_new = np.ascontiguousarray(np.concatenate(h_parts, axis=0), dtype=np.float32)
    c_new = np.ascontiguousarray(np.concatenate(c_parts, axis=0), dtype=np.float32)
    return h_new, c_new


def kernel(**inputs):
    inputs = {k: np.asarray(v) for k, v in inputs.items()}
    nc = _get_nc()
    in_maps = make_in_maps(**inputs)
    res = run_bass_kernel_spmd(nc, in_maps, list(range(N_CORES)))
    return unpack_results(res.results)

